# revision 74
# baseline (speedup 1.0000x reference)
"""Multi-head attention (B=8, N=1024, C=768, H=12, D=64) on 8 TRN2 NeuronCores.

Sharding: pure data parallel - one batch element per core, weights replicated,
no collectives. Each core computes its full attention block.

fp16 operands, fp32 PSUM. The attention phase is a strict producer/consumer
pipe between PE (scores) and ACT (exp). The scores tile for one m-tile is
[128, 2heads, 1024] fp32 = 4 PSUM banks, and with attn@v accumulators (2
banks) and the qk/proj accumulator ring (2 banks) resident, it cannot be
double-buffered in 8 banks - so exp(mi) -> scores(mi+1) -> exp(mi+1) is a
serial chain by construction (~0.9us of scores+semaphore latency inserted
per 2us exp). Finer-grained variants that do double-buffer (96 exps of FD
1024 in 2-bank tiles) were measured slower: per-op ACT overhead grows and
the PE's per-slot micro-idles trip the HAM clock gate into its 1.2GHz
state (cold matmuls at 427ns vs 216ns warm). The shipped design instead
paces all other PE work to the exp cadence so the PE queue never
head-of-line blocks and never idles long:
  - scores: both heads of a pair share one ps tile; their K=64 matmuls are
    issued adjacently with stationaries at base partitions 0/64, landing in
    disjoint PE row groups so the two streams run concurrently; one
    2048-wide exp per m-tile minimizes ACT per-op overhead.
  - all other PE work (attn@v chain steps, qk-projection n-halves, v
    projection head-halves, output-projection partials) is chopped into
    ~0.4-1.3us units and drained between exp slots by an emission-time
    credit pacer, so the in-order PE stream interleaves at sub-exp
    granularity (the static Tile scheduler alone interleaved too coarsely,
    costing ~1.5us/m-tile of PE idle plus HAM re-throttle).
  - attn@v stationary is [v | ones*64] (128 cols, M=128 costs the same
    cycles as M=65): the denominator lands in PSUM partitions 64:128
    already broadcast across 64 partitions, so normalization is a DVE-only
    chain (copy den, reciprocal_approx_fast on [64,512], tensor_mul
    directly from PSUM) with no PE broadcast matmuls and no ACT load.
  - exp: 1/sqrt(D) scale and a -2.0 bias folded into the ACTIVATE's free
    affine stage (softmax is shift-invariant).
  - proj bias and the A->B fp16-partial merge fold into the PSUM->SBUF hop
    as DVE tensor_adds (no identity matmuls).
  - head: batched >=0.8MB input DMAs on 3 queues (transfers only start
    ~8.4us in - fixed launch latency); ci-major PE transposes, 4 per PSUM
    bank with one batched copy per bank alternating DVE/ACT; a dummy
    accumulation chain trips the HAM clock gate out of its 1.2GHz idle
    state; the x arena is freed once the transposes complete.
  - tail: the last pair's attn@v runs dense from a fresh 6-bank pool; each
    chunk's norm overlaps the projection rows that only need the other
    chunk; tail den copies run on the then-idle Scalar engine; fp16 output
    DMA per row tile on alternating queues (fp16 is well inside the 2e-2
    gate and halves the output bytes).
"""

import numpy as np

B, N, C = 8, 1024, 768
H, D = 12, 64
F3 = 3 * C          # 2304
FQK = 2 * C         # 1536
SCALE = D ** -0.5   # 0.125
EXP_BIAS = -2.0     # exp(s*SCALE + EXP_BIAS); cancels in softmax
NT = N // 128       # 8 n-tiles / m-tiles
CT = C // 128       # 6 c-tiles
FT = FQK // 128     # 12 qk feature tiles
NCH = N // 512      # 2 psum chunks over n
VCH = 384           # v / proj free chunk (C = 2*384)
CP = CT // 2        # 3 c-pairs (onorm arena grouping)

_compiled = None


def _build():
    import concourse.mybir as mybir
    import concourse.tile as tile
    from concourse import bacc
    from concourse.masks import make_identity

    f32 = mybir.dt.float32
    f16 = mybir.dt.float16

    nc = bacc.Bacc("TRN2", target_bir_lowering=False, debug=False)

    x_d = nc.dram_tensor("x", [N, C], f16, kind="ExternalInput").ap()
    wqkv_d = nc.dram_tensor("w_qkv", [C, F3], f16, kind="ExternalInput").ap()
    wproj_d = nc.dram_tensor("w_proj", [C, C], f16, kind="ExternalInput").ap()
    bias_d = nc.dram_tensor("b_bcast", [128, C], f32, kind="ExternalInput").ap()
    out_d = nc.dram_tensor("out", [N, C], f16, kind="ExternalOutput").ap()

    with tile.TileContext(nc) as tc:
        with tc.tile_pool(name="const", bufs=1) as const_pool:
            ident_f32 = const_pool.tile([128, 128], f32)
            ident = const_pool.tile([128, 128], f16)
            bias_sb = const_pool.tile([128, C], f32)
            exp_bias = const_pool.tile([128, 1], f32)

            # ---- persistent activations ----
            with tc.tile_pool(name="acts", bufs=1) as acts:
                xT = [acts.tile([128, N], f16, tag=f"xT{ci}", name=f"xT{ci}")
                      for ci in range(CT)]
                qkT = [acts.tile([128, N], f16, tag=f"qkT{fi}", name=f"qkT{fi}")
                       for fi in range(FT)]
                # attn@v stationary: [p, mi, h, v|ones, d], m = mi*128 + p.
                # Columns 64:128 are constant 1.0 so the matmul deposits the
                # softmax denominator broadcast across PSUM partitions 64:128.
                vnat = acts.tile([128, NT, H, 2, D], f16, tag="vnat",
                                 name="vnat")
                # proj stationary: [p, cpair, j, n]; contraction
                # hd = cpair*256 + j*128 + p; head h lives at
                # [64*(h%2):64*(h%2)+64, h//4, (h//2)%2, :]
                onorm = acts.tile([128, CP, 2, N], f16, tag="onorm",
                                  name="onorm")

                with tc.tile_pool(name="wq", bufs=1) as wq_pool, \
                     tc.tile_pool(name="wp", bufs=1) as wp_pool:
                    # ---- phase 0: batched >=0.8MB input DMAs (small
                    # per-tile DMAs cap at ~75GB/s; 1MB reaches ~340GB/s) ----
                    acc_cm = tc.tile_pool(name="acc", bufs=2, space="PSUM")
                    acc_pool = acc_cm.__enter__()
                    xin_cm = tc.tile_pool(name="xin", bufs=1)
                    xin_pool = xin_cm.__enter__()
                    xt_a = xin_pool.tile([128, NT, C], f16, tag="xin",
                                         name="xin")
                    x_r = x_d.rearrange("(a p) c -> p a c", p=128)
                    make_identity(nc, ident_f32[:])
                    nc.vector.tensor_copy(ident[:], ident_f32[:])
                    nc.sync.dma_start(xt_a[:, 0:3, :], x_r[:, 0:3, :])
                    nc.gpsimd.dma_start(xt_a[:, 3:6, :], x_r[:, 3:6, :])
                    nc.scalar.dma_start(xt_a[:, 6:8, :], x_r[:, 6:8, :])
                    # dummy accumulation chain while DMAs stream: keeps the
                    # HAM clock gate from starting the kernel at 1.2GHz
                    wrm_cm = tc.tile_pool(name="wrm", bufs=1, space="PSUM")
                    wrm_pool = wrm_cm.__enter__()
                    wps = wrm_pool.tile([128, 128], f32, tag="w", name="wps")
                    for k in range(40):
                        nc.tensor.matmul(wps[:], ident[:], ident[:],
                                         start=(k == 0), stop=(k == 39))
                    wq_t = wq_pool.tile([128, CT, F3], f16, tag="wq",
                                        name="wq")
                    wq_r = wqkv_d.rearrange("(a p) f -> p a f", p=128)
                    nc.sync.dma_start(wq_t[:, 0:2, :], wq_r[:, 0:2, :])
                    nc.gpsimd.dma_start(wq_t[:, 2:4, :], wq_r[:, 2:4, :])
                    nc.scalar.dma_start(wq_t[:, 4:6, :], wq_r[:, 4:6, :])
                    wq = [wq_t[:, ci, :] for ci in range(CT)]
                    wp_t = wp_pool.tile([128, CT, C], f16, tag="wp",
                                        name="wp")
                    wp_r = wproj_d.rearrange("(a p) c -> p a c", p=128)
                    nc.sync.dma_start(wp_t[:, 0:3, :], wp_r[:, 0:3, :])
                    nc.gpsimd.dma_start(wp_t[:, 3:6, :], wp_r[:, 3:6, :])
                    wp = [wp_t[:, ci, :] for ci in range(CT)]
                    xt_ins = [xt_a[:, ni, :] for ni in range(NT)]
                    nc.gpsimd.dma_start(bias_sb[:], bias_d)
                    # ones columns of the attn@v stationary, written once
                    nc.gpsimd.memset(vnat[:, :, :, 1, :], 1.0)
                    nc.gpsimd.memset(exp_bias[:], EXP_BIAS)
                    def qk_half(fi, ch):
                        # one n-half of one qk feature tile: 6 MMs + 1 cast
                        pqk = acc_pool.tile([128, 512], f32, tag="acc",
                                            name=f"pqk{fi}_{ch}")
                        for ci in range(CT):
                            nc.tensor.matmul(
                                pqk[:],
                                wq[ci][:, fi * 128:(fi + 1) * 128],
                                xT[ci][:, ch * 512:(ch + 1) * 512],
                                start=(ci == 0), stop=(ci == CT - 1))
                        nc.vector.tensor_copy(
                            qkT[fi][:, ch * 512:(ch + 1) * 512], pqk[:])

                    # ci-major transposes: 4 per PSUM bank, one batched
                    # copy per bank alternating DVE/ACT; dummy-chain matmuls
                    # interleaved between groups keep the HAM clock gate warm
                    # (transposes do not count as PE activity)
                    with tc.tile_pool(name="ph0", bufs=2,
                                      space="PSUM") as ph0_pool:
                        for ci in range(CT):
                            for g in range(2):
                                ptg = ph0_pool.tile([128, 4, 128], f16,
                                                    tag="ptg",
                                                    name=f"ptg{ci}_{g}")
                                for k in range(4):
                                    ni = g * 4 + k
                                    nc.tensor.transpose(
                                        ptg[:, k, :],
                                        xt_ins[ni][:,
                                                   ci * 128:(ci + 1) * 128],
                                        ident[:])
                                cp = (nc.vector.tensor_copy
                                      if (ci * 2 + g) % 2 else nc.scalar.copy)
                                cp(xT[ci][:, g * 512:(g + 1) * 512],
                                   ptg[:].rearrange("p a b -> p (a b)"))
                            for k in range(3):
                                nc.tensor.matmul(wps[:], ident[:], ident[:],
                                                 start=(k == 0), stop=(k == 2))
                    wrm_cm.__exit__(None, None, None)
                    xin_cm.__exit__(None, None, None)
                    # head phase: q/k feature tiles for pair 0
                    for ch in range(NCH):
                        qk_half(0, ch)
                    for ch in range(NCH):
                        qk_half(6, ch)

                    def v_half(ni, vc):
                        # one head-half of one v row tile: 6 MMs + 1 cast
                        pv = acc_pool.tile([128, VCH], f32, tag="acc",
                                           name=f"pv{ni}_{vc}")
                        for ci in range(CT):
                            nc.tensor.matmul(
                                pv[:],
                                xT[ci][:, ni * 128:(ni + 1) * 128],
                                wq[ci][:, FQK + vc * VCH:
                                       FQK + (vc + 1) * VCH],
                                start=(ci == 0), stop=(ci == CT - 1))
                        nc.vector.tensor_copy(
                            vnat[:, ni, vc * 6:(vc + 1) * 6, 0, :],
                            pv[:].rearrange("p (h d) -> p h d", d=D))

                    # ---- attention: exp-paced software pipeline ----
                    attn_pools = (
                        tc.tile_pool(name="fin", bufs=1),
                        tc.tile_pool(name="rc", bufs=3),
                        tc.tile_pool(name="exp", bufs=8),
                        tc.tile_pool(name="pss", bufs=1, space="PSUM"),
                        tc.tile_pool(name="pso", bufs=2, space="PSUM"),
                    )
                    fin_pool, rc_pool, exp_pool, pss_pool, pso_pool = [
                        p.__enter__() for p in attn_pools]

                    def norm(h, ch, po_h, den_on_act):
                        # po_h[0:64]  = unnormalized attn@v out [d, n-chunk]
                        # po_h[64:128] = denominator, already broadcast
                        den = rc_pool.tile([64, 512], f32, tag="den",
                                           name=f"den{h}_{ch}", bufs=3)
                        (nc.scalar.copy if den_on_act
                         else nc.vector.tensor_copy)(den[:], po_h[64:128, :])
                        rcf = rc_pool.tile([64, 512], f32, tag="rcf",
                                           name=f"rcf{h}_{ch}", bufs=3)
                        nc.vector.reciprocal_approx_fast(rcf[:], den[:])
                        s = h // 2
                        nc.vector.tensor_mul(
                            onorm[64 * (h % 2):64 * (h % 2) + D,
                                  s // 2, s % 2,
                                  ch * 512:(ch + 1) * 512],
                            po_h[0:64, :], rcf[:])

                    finA = {}

                    def proj_a_half(ni, fc):
                        # head pairs 0-3 (cpairs 0-1), one VCH half; bias
                        # folds into the PSUM->SBUF hop as a DVE tensor_add
                        pf = acc_pool.tile([128, VCH], f32, tag="acc",
                                           name=f"pfa{ni}_{fc}")
                        for cp in range(2):
                            for jj in range(2):
                                nc.tensor.matmul(
                                    pf[:],
                                    onorm[:, cp, jj,
                                          ni * 128:(ni + 1) * 128],
                                    wp[cp * 2 + jj][:,
                                        fc * VCH:(fc + 1) * VCH],
                                    start=(cp == 0 and jj == 0),
                                    stop=(cp == 1 and jj == 1))
                        if ni not in finA:
                            finA[ni] = fin_pool.tile(
                                [128, C], f16, tag="finA",
                                name=f"finA{ni}", bufs=8)
                        sl = slice(fc * VCH, (fc + 1) * VCH)
                        nc.vector.tensor_add(finA[ni][:, sl], pf[:],
                                             bias_sb[:, sl])

                    def proj_b(ni, po_pool):
                        # heads 8-11 (cp2); the accumulated fp16 partial
                        # (which already carries the bias) folds in via the
                        # PSUM->SBUF hop as a DVE tensor_add
                        pf = [po_pool.tile([128, VCH], f32, tag="po",
                                           name=f"pfb{ni}_{fc}")
                              for fc in range(2)]
                        for fc in range(2):
                            for jj in range(2):
                                nc.tensor.matmul(
                                    pf[fc][:],
                                    onorm[:, 2, jj, ni * 128:(ni + 1) * 128],
                                    wp[4 + jj][:, fc * VCH:(fc + 1) * VCH],
                                    start=(jj == 0), stop=(jj == 1))
                        fin = fin_pool.tile([128, C], f16, tag="fin",
                                            name=f"fin{ni}", bufs=2)
                        for fc in range(2):
                            sl = slice(fc * VCH, (fc + 1) * VCH)
                            nc.vector.tensor_add(fin[:, sl], pf[fc][:],
                                                 finA[ni][:, sl])
                        eng = nc.sync if ni % 2 == 0 else nc.gpsimd
                        eng.dma_start(
                            out_d[ni * 128:(ni + 1) * 128, :], fin[:])

                    # ---- filler units: (cost_ns, emit_fn), drained between
                    # score chunks to pace PE work to the exp cadence ----
                    def mk_attnv_units(jp, exp_t):
                        # attn@v chains for pair jp, one ch at a time so the
                        # pso ring (2 slots) recycles between chains; norms
                        # emitted right after each chain so slots free early
                        pair = (2 * jp, 2 * jp + 1)
                        units = []
                        for ch in range(NCH):
                            po = {}

                            def step(s, ch=ch, po=po, pair=pair,
                                     exp_t=exp_t):
                                for h in pair:
                                    if s == 0:
                                        po[h] = pso_pool.tile(
                                            [128, 512], f32, tag="po",
                                            name=f"po{jp}_{h}_{ch}")
                                    nc.tensor.matmul(
                                        po[h][:],
                                        vnat[:, s, h, :, :],
                                        exp_read(exp_t[s // 2], s, ch, h),
                                        start=(s == 0), stop=(s == NT - 1))

                            for s in range(NT):
                                units.append((440, lambda s=s, f=step: f(s)))
                            for h in pair:
                                units.append(
                                    (80, lambda h=h, ch=ch, po=po:
                                     norm(h, ch, po[h][:],
                                          den_on_act=False)))
                        return units

                    def exp_read(et, mi, ch, h):
                        return et[:, h % 2, mi % 2,
                                  ch * 512:(ch + 1) * 512]

                    def interleave(primary, secondary):
                        # 2 primary : 1 secondary, preserving order
                        out = []
                        p, s = list(primary), list(secondary)
                        while p or s:
                            for _ in range(2):
                                if p:
                                    out.append(p.pop(0))
                            if s:
                                out.append(s.pop(0))
                        return out

                    exp_ts = {}
                    SLOT_CREDIT = 2000  # ~exp duration; fillers pace to it

                    for j in range(H // 2):
                        pair = (2 * j, 2 * j + 1)
                        exp_ts[j] = []
                        exp_t = exp_ts[j]

                        fillers = []
                        if j + 1 < H // 2:
                            fillers += [(1300, lambda fi=fi, ch=ch:
                                         qk_half(fi, ch))
                                        for fi in (j + 1, 7 + j)
                                        for ch in range(NCH)]
                        if j == 0:
                            fillers += [(1000, lambda ni=ni: v_half(ni, 0))
                                        for ni in range(NT)]
                        sec = []
                        if 1 <= j <= 3:
                            # v heads 6-11, needed first by pair 3 at j=4
                            sec += [(1000, lambda ni=ni: v_half(ni, 1))
                                    for ni in range(NT)
                                    if ni % 3 == (j - 1) % 3]
                        if j == 4:
                            sec += [(700, lambda ni=ni, fc=fc:
                                     proj_a_half(ni, fc))
                                    for ni in range(2) for fc in range(2)]
                        if j == 5:
                            sec += [(700, lambda ni=ni, fc=fc:
                                     proj_a_half(ni, fc))
                                    for ni in range(2, NT) for fc in range(2)]
                        if j >= 1:
                            fillers = interleave(
                                mk_attnv_units(j - 1, exp_ts.pop(j - 1)),
                                fillers + sec)
                        else:
                            fillers += sec


                        credit = 0
                        for mi in range(NT):
                            if mi % 2 == 0:
                                et = exp_pool.tile([128, 2, 2, N], f16,
                                                   tag="exp",
                                                   name=f"exp{j}_{mi // 2}")
                                exp_t.append(et)
                            ps = pss_pool.tile([128, 2, N], f32, tag="pss",
                                               name=f"ps{j}_{mi}")
                            for ch in range(NCH):
                                for idx, h in enumerate(pair):
                                    qrow = (h % 2) * D
                                    nc.tensor.matmul(
                                        ps[:, idx,
                                           ch * 512:(ch + 1) * 512],
                                        qkT[6 + j][qrow:qrow + D,
                                                   mi * 128:(mi + 1) * 128],
                                        qkT[j][qrow:qrow + D,
                                               ch * 512:(ch + 1) * 512],
                                        start=True, stop=True)
                            nc.scalar.activation(
                                exp_t[mi // 2][:, :, mi % 2, :], ps[:],
                                mybir.ActivationFunctionType.Exp,
                                bias=exp_bias[:], scale=SCALE)
                            # pace filler work to the exp cadence
                            credit = SLOT_CREDIT
                            while fillers and credit > 0:
                                cost, fn = fillers.pop(0)
                                fn()
                                credit -= cost
                        # drain any leftover units before the next pair
                        for cost, fn in fillers:
                            fn()

                    # last pair: scores PSUM banks are free now; run its
                    # attn@v from a fresh pool so it never waits on norm-slot
                    # recycling
                    attn_pools[4].__exit__(None, None, None)
                    attn_pools[3].__exit__(None, None, None)
                    with tc.tile_pool(name="tail", bufs=6,
                                      space="PSUM") as tail_pool:
                        # the last pair: both attn@v chunk chains dense, then
                        # per chunk: norm, then the proj rows that only need
                        # that chunk's columns
                        pair = (H - 2, H - 1)
                        exp_t = exp_ts.pop(H // 2 - 1)
                        po = {}
                        for ch in range(NCH):
                            for h in pair:
                                po[h, ch] = tail_pool.tile(
                                    [128, 512], f32, tag="po",
                                    name=f"tpo{h}_{ch}")
                            for mi in range(NT):
                                for h in pair:
                                    nc.tensor.matmul(
                                        po[h, ch][:],
                                        vnat[:, mi, h, :, :],
                                        exp_read(exp_t[mi // 2], mi, ch, h),
                                        start=(mi == 0), stop=(mi == NT - 1))
                        for ch in range(NCH):
                            for h in pair:
                                norm(h, ch, po[h, ch][:], den_on_act=True)
                            for ni in range(ch * 4, ch * 4 + 4):
                                proj_b(ni, tail_pool)

                    for p in (attn_pools[2], attn_pools[1], attn_pools[0]):
                        p.__exit__(None, None, None)
                    acc_cm.__exit__(None, None, None)

    nc.compile()
    return nc


def _get_compiled():
    global _compiled
    if _compiled is None:
        _compiled = _build()
    return _compiled


def _run(x, w_qkv, w_proj, b_proj, **kwargs):
    from concourse.bass_utils import run_bass_kernel_spmd

    x = np.asarray(x, dtype=np.float32).astype(np.float16)
    w_qkv = np.ascontiguousarray(
        np.asarray(w_qkv, dtype=np.float32).astype(np.float16))
    w_proj = np.ascontiguousarray(
        np.asarray(w_proj, dtype=np.float32).astype(np.float16))
    b_bcast = np.ascontiguousarray(
        np.broadcast_to(np.asarray(b_proj, dtype=np.float32), (128, C)))

    nc = _get_compiled()
    in_maps = [
        {"x": np.ascontiguousarray(x[b]), "w_qkv": w_qkv,
         "w_proj": w_proj, "b_bcast": b_bcast}
        for b in range(B)
    ]
    return run_bass_kernel_spmd(nc, in_maps, core_ids=list(range(B)), **kwargs)


def kernel(x, w_qkv, w_proj, b_proj, **_):
    res = _run(x, w_qkv, w_proj, b_proj)
    return np.stack(
        [res.results[b]["out"] for b in range(B)], axis=0
    ).astype(np.float32)


# revision 75
# speedup vs baseline: 1.0098x; 1.0098x over previous
"""Multi-head attention (B=8, N=1024, C=768, H=12, D=64) on 8 TRN2 NeuronCores.

Sharding: pure data parallel - one batch element per core, weights replicated,
no collectives. Each core computes its full attention block.

fp16 operands, fp32 PSUM. The attention phase is a strict producer/consumer
pipe between PE (scores) and ACT (exp). The scores tile for one m-tile is
[128, 2heads, 1024] fp32 = 4 PSUM banks, and with attn@v accumulators (2
banks) and the qk/proj accumulator ring (2 banks) resident, it cannot be
double-buffered in 8 banks - so exp(mi) -> scores(mi+1) -> exp(mi+1) is a
serial chain by construction (~0.9us of scores+semaphore latency inserted
per 2us exp). Finer-grained variants that do double-buffer (96 exps of FD
1024 in 2-bank tiles) were measured slower: per-op ACT overhead grows and
the PE's per-slot micro-idles trip the HAM clock gate into its 1.2GHz
state (cold matmuls at 427ns vs 216ns warm). The shipped design instead
paces all other PE work to the exp cadence so the PE queue never
head-of-line blocks and never idles long:
  - scores: both heads of a pair share one ps tile; their K=64 matmuls are
    issued adjacently with stationaries at base partitions 0/64, landing in
    disjoint PE row groups so the two streams run concurrently; one
    2048-wide exp per m-tile minimizes ACT per-op overhead.
  - all other PE work (attn@v chain steps, qk-projection n-halves, v
    projection head-halves, output-projection partials) is chopped into
    ~0.4-1.3us units and drained between exp slots by an emission-time
    credit pacer, so the in-order PE stream interleaves at sub-exp
    granularity (the static Tile scheduler alone interleaved too coarsely,
    costing ~1.5us/m-tile of PE idle plus HAM re-throttle).
  - attn@v stationary is [v | ones*64] (128 cols, M=128 costs the same
    cycles as M=65): the denominator lands in PSUM partitions 64:128
    already broadcast across 64 partitions, so normalization is a DVE-only
    chain (copy den, reciprocal_approx_fast on [64,512], tensor_mul
    directly from PSUM) with no PE broadcast matmuls and no ACT load.
  - exp: 1/sqrt(D) scale and a -2.0 bias folded into the ACTIVATE's free
    affine stage (softmax is shift-invariant).
  - proj bias and the A->B fp16-partial merge fold into the PSUM->SBUF hop
    as DVE tensor_adds (no identity matmuls).
  - head: batched >=0.8MB input DMAs on 3 queues (transfers only start
    ~8.4us in - fixed launch latency); ci-major PE transposes, 4 per PSUM
    bank with one batched copy per bank alternating DVE/ACT; a dummy
    accumulation chain trips the HAM clock gate out of its 1.2GHz idle
    state; the x arena is freed once the transposes complete.
  - tail: the last pair's attn@v runs dense from a fresh 6-bank pool; each
    chunk's norm overlaps the projection rows that only need the other
    chunk; tail den copies run on the then-idle Scalar engine; fp16 output
    DMA per row tile on alternating queues (fp16 is well inside the 2e-2
    gate and halves the output bytes).
"""

import numpy as np

B, N, C = 8, 1024, 768
H, D = 12, 64
F3 = 3 * C          # 2304
FQK = 2 * C         # 1536
SCALE = D ** -0.5   # 0.125
EXP_BIAS = -2.0     # exp(s*SCALE + EXP_BIAS); cancels in softmax
NT = N // 128       # 8 n-tiles / m-tiles
CT = C // 128       # 6 c-tiles
FT = FQK // 128     # 12 qk feature tiles
NCH = N // 512      # 2 psum chunks over n
VCH = 384           # v / proj free chunk (C = 2*384)
CP = CT // 2        # 3 c-pairs (onorm arena grouping)

_compiled = None


def _build():
    import concourse.mybir as mybir
    import concourse.tile as tile
    from concourse import bacc
    from concourse.masks import make_identity

    f32 = mybir.dt.float32
    f16 = mybir.dt.float16

    nc = bacc.Bacc("TRN2", target_bir_lowering=False, debug=False)

    x_d = nc.dram_tensor("x", [N, C], f16, kind="ExternalInput").ap()
    wqkv_d = nc.dram_tensor("w_qkv", [C, F3], f16, kind="ExternalInput").ap()
    wproj_d = nc.dram_tensor("w_proj", [C, C], f16, kind="ExternalInput").ap()
    bias_d = nc.dram_tensor("b_bcast", [128, C], f32, kind="ExternalInput").ap()
    out_d = nc.dram_tensor("out", [N, C], f16, kind="ExternalOutput").ap()

    with tile.TileContext(nc) as tc:
        with tc.tile_pool(name="const", bufs=1) as const_pool:
            ident_f32 = const_pool.tile([128, 128], f32)
            ident = const_pool.tile([128, 128], f16)
            bias_sb = const_pool.tile([128, C], f32)
            exp_bias = const_pool.tile([128, 1], f32)

            # ---- persistent activations ----
            with tc.tile_pool(name="acts", bufs=1) as acts:
                xT = [acts.tile([128, N], f16, tag=f"xT{ci}", name=f"xT{ci}")
                      for ci in range(CT)]
                qkT = [acts.tile([128, N], f16, tag=f"qkT{fi}", name=f"qkT{fi}")
                       for fi in range(FT)]
                # attn@v stationary: [p, mi, h, v|ones, d], m = mi*128 + p.
                # Columns 64:128 are constant 1.0 so the matmul deposits the
                # softmax denominator broadcast across PSUM partitions 64:128.
                vnat = acts.tile([128, NT, H, 2, D], f16, tag="vnat",
                                 name="vnat")
                # proj stationary: [p, cpair, j, n]; contraction
                # hd = cpair*256 + j*128 + p; head h lives at
                # [64*(h%2):64*(h%2)+64, h//4, (h//2)%2, :]
                onorm = acts.tile([128, CP, 2, N], f16, tag="onorm",
                                  name="onorm")

                with tc.tile_pool(name="wq", bufs=1) as wq_pool, \
                     tc.tile_pool(name="wp", bufs=1) as wp_pool:
                    # ---- phase 0: batched >=0.8MB input DMAs (small
                    # per-tile DMAs cap at ~75GB/s; 1MB reaches ~340GB/s) ----
                    acc_cm = tc.tile_pool(name="acc", bufs=2, space="PSUM")
                    acc_pool = acc_cm.__enter__()
                    xin_cm = tc.tile_pool(name="xin", bufs=1)
                    xin_pool = xin_cm.__enter__()
                    xt_a = xin_pool.tile([128, NT, C], f16, tag="xin",
                                         name="xin")
                    x_r = x_d.rearrange("(a p) c -> p a c", p=128)
                    make_identity(nc, ident_f32[:])
                    nc.vector.tensor_copy(ident[:], ident_f32[:])
                    nc.sync.dma_start(xt_a[:, 0:3, :], x_r[:, 0:3, :])
                    nc.gpsimd.dma_start(xt_a[:, 3:6, :], x_r[:, 3:6, :])
                    nc.scalar.dma_start(xt_a[:, 6:8, :], x_r[:, 6:8, :])
                    # dummy accumulation chain while DMAs stream: keeps the
                    # HAM clock gate from starting the kernel at 1.2GHz
                    wrm_cm = tc.tile_pool(name="wrm", bufs=1, space="PSUM")
                    wrm_pool = wrm_cm.__enter__()
                    wps = wrm_pool.tile([128, 128], f32, tag="w", name="wps")
                    for k in range(40):
                        nc.tensor.matmul(wps[:], ident[:], ident[:],
                                         start=(k == 0), stop=(k == 39))
                    wq_t = wq_pool.tile([128, CT, F3], f16, tag="wq",
                                        name="wq")
                    wq_r = wqkv_d.rearrange("(a p) f -> p a f", p=128)
                    nc.sync.dma_start(wq_t[:, 0:2, :], wq_r[:, 0:2, :])
                    nc.gpsimd.dma_start(wq_t[:, 2:4, :], wq_r[:, 2:4, :])
                    nc.scalar.dma_start(wq_t[:, 4:6, :], wq_r[:, 4:6, :])
                    wq = [wq_t[:, ci, :] for ci in range(CT)]
                    wp_t = wp_pool.tile([128, CT, C], f16, tag="wp",
                                        name="wp")
                    wp_r = wproj_d.rearrange("(a p) c -> p a c", p=128)
                    nc.sync.dma_start(wp_t[:, 0:3, :], wp_r[:, 0:3, :])
                    nc.gpsimd.dma_start(wp_t[:, 3:6, :], wp_r[:, 3:6, :])
                    wp = [wp_t[:, ci, :] for ci in range(CT)]
                    xt_ins = [xt_a[:, ni, :] for ni in range(NT)]
                    nc.gpsimd.dma_start(bias_sb[:], bias_d)
                    # ones columns of the attn@v stationary, written once
                    nc.gpsimd.memset(vnat[:, :, :, 1, :], 1.0)
                    nc.gpsimd.memset(exp_bias[:], EXP_BIAS)
                    def qk_half(fi, ch):
                        # one n-half of one qk feature tile: 6 MMs + 1 cast
                        pqk = acc_pool.tile([128, 512], f32, tag="acc",
                                            name=f"pqk{fi}_{ch}")
                        for ci in range(CT):
                            nc.tensor.matmul(
                                pqk[:],
                                wq[ci][:, fi * 128:(fi + 1) * 128],
                                xT[ci][:, ch * 512:(ch + 1) * 512],
                                start=(ci == 0), stop=(ci == CT - 1))
                        nc.vector.tensor_copy(
                            qkT[fi][:, ch * 512:(ch + 1) * 512], pqk[:])

                    # ci-major transposes: 4 per PSUM bank, one batched
                    # copy per bank alternating DVE/ACT; dummy-chain matmuls
                    # interleaved between groups keep the HAM clock gate warm
                    # (transposes do not count as PE activity)
                    with tc.tile_pool(name="ph0", bufs=2,
                                      space="PSUM") as ph0_pool:
                        for ci in range(CT):
                            for g in range(2):
                                ptg = ph0_pool.tile([128, 4, 128], f16,
                                                    tag="ptg",
                                                    name=f"ptg{ci}_{g}")
                                for k in range(4):
                                    ni = g * 4 + k
                                    nc.tensor.transpose(
                                        ptg[:, k, :],
                                        xt_ins[ni][:,
                                                   ci * 128:(ci + 1) * 128],
                                        ident[:])
                                cp = (nc.vector.tensor_copy
                                      if (ci * 2 + g) % 2 else nc.scalar.copy)
                                cp(xT[ci][:, g * 512:(g + 1) * 512],
                                   ptg[:].rearrange("p a b -> p (a b)"))
                            for k in range(3):
                                nc.tensor.matmul(wps[:], ident[:], ident[:],
                                                 start=(k == 0), stop=(k == 2))
                    wrm_cm.__exit__(None, None, None)
                    xin_cm.__exit__(None, None, None)
                    # head phase: q/k feature tiles for pair 0
                    for ch in range(NCH):
                        qk_half(0, ch)
                    for ch in range(NCH):
                        qk_half(6, ch)

                    def v_half(ni, vc):
                        # one head-half of one v row tile: 6 MMs + 1 cast
                        pv = acc_pool.tile([128, VCH], f32, tag="acc",
                                           name=f"pv{ni}_{vc}")
                        for ci in range(CT):
                            nc.tensor.matmul(
                                pv[:],
                                xT[ci][:, ni * 128:(ni + 1) * 128],
                                wq[ci][:, FQK + vc * VCH:
                                       FQK + (vc + 1) * VCH],
                                start=(ci == 0), stop=(ci == CT - 1))
                        nc.vector.tensor_copy(
                            vnat[:, ni, vc * 6:(vc + 1) * 6, 0, :],
                            pv[:].rearrange("p (h d) -> p h d", d=D))

                    # ---- attention: exp-paced software pipeline ----
                    attn_pools = (
                        tc.tile_pool(name="fin", bufs=1),
                        tc.tile_pool(name="rc", bufs=2),
                        tc.tile_pool(name="exp", bufs=8),
                        tc.tile_pool(name="pss", bufs=1, space="PSUM"),
                        tc.tile_pool(name="pso", bufs=2, space="PSUM"),
                    )
                    fin_pool, rc_pool, exp_pool, pss_pool, pso_pool = [
                        p.__enter__() for p in attn_pools]

                    def norm(h, ch, po_h, den_on_act):
                        # po_h[0:64]  = unnormalized attn@v out [d, n-chunk]
                        # po_h[64:128] = denominator, already broadcast
                        den = rc_pool.tile([64, 512], f32, tag="den",
                                           name=f"den{h}_{ch}", bufs=2)
                        (nc.scalar.copy if den_on_act
                         else nc.vector.tensor_copy)(den[:], po_h[64:128, :])
                        rcf = rc_pool.tile([64, 512], f32, tag="rcf",
                                           name=f"rcf{h}_{ch}", bufs=2)
                        nc.vector.reciprocal_approx_fast(rcf[:], den[:])
                        s = h // 2
                        nc.vector.tensor_mul(
                            onorm[64 * (h % 2):64 * (h % 2) + D,
                                  s // 2, s % 2,
                                  ch * 512:(ch + 1) * 512],
                            po_h[0:64, :], rcf[:])

                    finA = {}

                    def proj_a_half(ni, fc):
                        # head pairs 0-3 (cpairs 0-1), one VCH half; bias
                        # folds into the PSUM->SBUF hop as a DVE tensor_add
                        pf = acc_pool.tile([128, VCH], f32, tag="acc",
                                           name=f"pfa{ni}_{fc}")
                        for cp in range(2):
                            for jj in range(2):
                                nc.tensor.matmul(
                                    pf[:],
                                    onorm[:, cp, jj,
                                          ni * 128:(ni + 1) * 128],
                                    wp[cp * 2 + jj][:,
                                        fc * VCH:(fc + 1) * VCH],
                                    start=(cp == 0 and jj == 0),
                                    stop=(cp == 1 and jj == 1))
                        if ni not in finA:
                            finA[ni] = fin_pool.tile(
                                [128, C], f16, tag="finA",
                                name=f"finA{ni}", bufs=8)
                        sl = slice(fc * VCH, (fc + 1) * VCH)
                        nc.vector.tensor_add(finA[ni][:, sl], pf[:],
                                             bias_sb[:, sl])

                    def proj_b(ni, po_pool):
                        # heads 8-11 (cp2); the accumulated fp16 partial
                        # (which already carries the bias) folds in via the
                        # PSUM->SBUF hop as a DVE tensor_add
                        pf = [po_pool.tile([128, VCH], f32, tag="po",
                                           name=f"pfb{ni}_{fc}")
                              for fc in range(2)]
                        for fc in range(2):
                            for jj in range(2):
                                nc.tensor.matmul(
                                    pf[fc][:],
                                    onorm[:, 2, jj, ni * 128:(ni + 1) * 128],
                                    wp[4 + jj][:, fc * VCH:(fc + 1) * VCH],
                                    start=(jj == 0), stop=(jj == 1))
                        fin = fin_pool.tile([128, C], f16, tag="fin",
                                            name=f"fin{ni}", bufs=2)
                        for fc in range(2):
                            sl = slice(fc * VCH, (fc + 1) * VCH)
                            nc.vector.tensor_add(fin[:, sl], pf[fc][:],
                                                 finA[ni][:, sl])
                        eng = nc.sync if ni % 2 == 0 else nc.gpsimd
                        eng.dma_start(
                            out_d[ni * 128:(ni + 1) * 128, :], fin[:])

                    # ---- filler units: (cost_ns, emit_fn), drained between
                    # score chunks to pace PE work to the exp cadence ----
                    def mk_attnv_units(jp, exp_t):
                        # attn@v chains for pair jp, one ch at a time so the
                        # pso ring (2 slots) recycles between chains; norms
                        # emitted right after each chain so slots free early
                        pair = (2 * jp, 2 * jp + 1)
                        units = []
                        for ch in range(NCH):
                            po = {}

                            def step(s, ch=ch, po=po, pair=pair,
                                     exp_t=exp_t):
                                for h in pair:
                                    if s == 0:
                                        po[h] = pso_pool.tile(
                                            [128, 512], f32, tag="po",
                                            name=f"po{jp}_{h}_{ch}")
                                    nc.tensor.matmul(
                                        po[h][:],
                                        vnat[:, s, h, :, :],
                                        exp_read(exp_t[s // 2], s, ch, h),
                                        start=(s == 0), stop=(s == NT - 1))

                            for s in range(NT):
                                units.append((440, lambda s=s, f=step: f(s)))
                            for h in pair:
                                units.append(
                                    (80, lambda h=h, ch=ch, po=po:
                                     norm(h, ch, po[h][:],
                                          den_on_act=False)))
                        return units

                    def exp_read(et, mi, ch, h):
                        return et[:, h % 2, mi % 2,
                                  ch * 512:(ch + 1) * 512]

                    def interleave(primary, secondary):
                        # 2 primary : 1 secondary, preserving order
                        out = []
                        p, s = list(primary), list(secondary)
                        while p or s:
                            for _ in range(2):
                                if p:
                                    out.append(p.pop(0))
                            if s:
                                out.append(s.pop(0))
                        return out

                    exp_ts = {}
                    SLOT_CREDIT = 1900  # ~exp duration; fillers pace to it

                    for j in range(H // 2):
                        pair = (2 * j, 2 * j + 1)
                        exp_ts[j] = []
                        exp_t = exp_ts[j]

                        fillers = []
                        if j + 1 < H // 2:
                            fillers += [(1300, lambda fi=fi, ch=ch:
                                         qk_half(fi, ch))
                                        for fi in (j + 1, 7 + j)
                                        for ch in range(NCH)]
                        if j == 0:
                            fillers += [(1000, lambda ni=ni: v_half(ni, 0))
                                        for ni in range(NT)]
                        sec = []
                        if 1 <= j <= 3:
                            # v heads 6-11, needed first by pair 3 at j=4
                            sec += [(1000, lambda ni=ni: v_half(ni, 1))
                                    for ni in range(NT)
                                    if ni % 3 == (j - 1) % 3]
                        if j == 4:
                            sec += [(700, lambda ni=ni, fc=fc:
                                     proj_a_half(ni, fc))
                                    for ni in range(2) for fc in range(2)]
                        if j == 5:
                            sec += [(700, lambda ni=ni, fc=fc:
                                     proj_a_half(ni, fc))
                                    for ni in range(2, 6) for fc in range(2)]
                        if j >= 1:
                            fillers = interleave(
                                mk_attnv_units(j - 1, exp_ts.pop(j - 1)),
                                fillers + sec)
                        else:
                            fillers += sec


                        credit = 0
                        for mi in range(NT):
                            if mi % 2 == 0:
                                et = exp_pool.tile([128, 2, 2, N], f16,
                                                   tag="exp",
                                                   name=f"exp{j}_{mi // 2}")
                                exp_t.append(et)
                            ps = pss_pool.tile([128, 2, N], f32, tag="pss",
                                               name=f"ps{j}_{mi}")
                            for ch in range(NCH):
                                for idx, h in enumerate(pair):
                                    qrow = (h % 2) * D
                                    nc.tensor.matmul(
                                        ps[:, idx,
                                           ch * 512:(ch + 1) * 512],
                                        qkT[6 + j][qrow:qrow + D,
                                                   mi * 128:(mi + 1) * 128],
                                        qkT[j][qrow:qrow + D,
                                               ch * 512:(ch + 1) * 512],
                                        start=True, stop=True)
                            nc.scalar.activation(
                                exp_t[mi // 2][:, :, mi % 2, :], ps[:],
                                mybir.ActivationFunctionType.Exp,
                                bias=exp_bias[:], scale=SCALE)
                            # pace filler work to the exp cadence
                            credit = SLOT_CREDIT
                            while fillers and credit > 0:
                                cost, fn = fillers.pop(0)
                                fn()
                                credit -= cost
                        # drain any leftover units before the next pair
                        for cost, fn in fillers:
                            fn()

                    # last pair: scores PSUM banks are free now; run its
                    # attn@v from a fresh pool so it never waits on norm-slot
                    # recycling
                    attn_pools[4].__exit__(None, None, None)
                    attn_pools[3].__exit__(None, None, None)
                    with tc.tile_pool(name="tail", bufs=6,
                                      space="PSUM") as tail_pool:
                        # remaining proj_a rows first (their onorm inputs are
                        # long ready), then the last pair: both attn@v chunk
                        # chains dense, then per chunk: norm, then the proj
                        # rows that only need that chunk's columns
                        for ni in range(6, NT):
                            for fc in range(2):
                                proj_a_half(ni, fc)
                        pair = (H - 2, H - 1)
                        exp_t = exp_ts.pop(H // 2 - 1)
                        po = {}
                        for ch in range(NCH):
                            for h in pair:
                                po[h, ch] = tail_pool.tile(
                                    [128, 512], f32, tag="po",
                                    name=f"tpo{h}_{ch}")
                            for mi in range(NT):
                                for h in pair:
                                    nc.tensor.matmul(
                                        po[h, ch][:],
                                        vnat[:, mi, h, :, :],
                                        exp_read(exp_t[mi // 2], mi, ch, h),
                                        start=(mi == 0), stop=(mi == NT - 1))
                        for ch in range(NCH):
                            for h in pair:
                                norm(h, ch, po[h, ch][:], den_on_act=True)
                            for ni in range(ch * 4, ch * 4 + 4):
                                proj_b(ni, tail_pool)

                    for p in (attn_pools[2], attn_pools[1], attn_pools[0]):
                        p.__exit__(None, None, None)
                    acc_cm.__exit__(None, None, None)

    nc.compile()
    return nc


def _get_compiled():
    global _compiled
    if _compiled is None:
        _compiled = _build()
    return _compiled


def _run(x, w_qkv, w_proj, b_proj, **kwargs):
    from concourse.bass_utils import run_bass_kernel_spmd

    x = np.asarray(x, dtype=np.float32).astype(np.float16)
    w_qkv = np.ascontiguousarray(
        np.asarray(w_qkv, dtype=np.float32).astype(np.float16))
    w_proj = np.ascontiguousarray(
        np.asarray(w_proj, dtype=np.float32).astype(np.float16))
    b_bcast = np.ascontiguousarray(
        np.broadcast_to(np.asarray(b_proj, dtype=np.float32), (128, C)))

    nc = _get_compiled()
    in_maps = [
        {"x": np.ascontiguousarray(x[b]), "w_qkv": w_qkv,
         "w_proj": w_proj, "b_bcast": b_bcast}
        for b in range(B)
    ]
    return run_bass_kernel_spmd(nc, in_maps, core_ids=list(range(B)), **kwargs)


def kernel(x, w_qkv, w_proj, b_proj, **_):
    res = _run(x, w_qkv, w_proj, b_proj)
    return np.stack(
        [res.results[b]["out"] for b in range(B)], axis=0
    ).astype(np.float32)
